# revision 1
# baseline (speedup 1.0000x reference)
"""MeshConvPoint Trainium2 kernel (8-core SPMD).

Math: per vertex v with gathered features f0..f3 (4 random indices/vertex):
  G = [f0, p1, e3, e2, p2, 2*(mx-mn), p3] channelwise over f1..f3,
  out = einsum(G, W) + b
where p_k = sum_j f_j^k, e2=(p1^2-p2)/2, e3=(p1^3-3 p1 p2 + 2 p3)/6,
mx/mn = max/min over f1..f3.  All symmetric functions reduce to the
8 features [f0, p1, p2, p3, p1^2, p1^3, p1*p2, mx-mn]; the linear
recombination + scale factors are folded into the weights host-side, so the
device does: gather -> power sums -> PE transposes -> one 512-contract
matmul per 512-vertex supertile.

Sharding: 8 cores = 4 batches x 2 vertex halves (data parallel; each core
holds the full per-batch gather table, so no collectives).

Gather: rows fetched with indirect DMA from a [V, 64] f32-typed table whose
payload is bit-packed fp16 [x | x^2] (squares ride along for free since the
cost is per-descriptor, not per-byte).
"""

import sys

sys.path.insert(0, "/opt/trn_rl_repo")

import numpy as np

import concourse.bass as bass
import concourse.tile as tile
from concourse import bacc, mybir
from concourse.bass_utils import run_bass_kernel_spmd
from concourse.masks import make_identity

B, C, V, CO, K = 4, 64, 50000, 128, 7
VPC = 25088          # padded vertices per core (2 halves of 50000 -> 196*128)
SG = 512             # supertile = 4 vtiles of 128 vertices
NST = VPC // SG      # 49 supertiles
TBLR = 32768         # fixed compacted table rows (unique refs per slot-pair < 32768)
CALLV = 1024         # vertices per dma_gather call (2 supertiles)
F16 = mybir.dt.float16
F32 = mybir.dt.float32
I32 = mybir.dt.int32

_cache = {}


def build_program(loop_iters=1, nst=NST, two_phase=True):
    key = (loop_iters, nst, two_phase)
    if key in _cache:
        return _cache[key]
    nc = bacc.Bacc("TRN2", target_bir_lowering=False, debug=False, num_devices=8)
    tblA = nc.dram_tensor("tblA", [TBLR, C], F32, kind="ExternalInput").ap()
    tblB = nc.dram_tensor("tblB", [TBLR, C], F32, kind="ExternalInput").ap()
    idxA = nc.dram_tensor("idxA", [128, VPC * 2 // 16], mybir.dt.int16, kind="ExternalInput").ap()
    idxB = nc.dram_tensor("idxB", [128, VPC * 2 // 16], mybir.dt.int16, kind="ExternalInput").ap()
    wch = nc.dram_tensor("wch", [4, 128, 128], F16, kind="ExternalInput").ap()
    bias = nc.dram_tensor("bias", [128, 1], F32, kind="ExternalInput").ap()
    out = nc.dram_tensor("out", [128, nst * SG], F16, kind="ExternalOutput").ap()

    with tile.TileContext(nc) as tc:
        import contextlib

        with contextlib.ExitStack() as ctx:
            cst = ctx.enter_context(tc.tile_pool(name="cst", bufs=1))
            gpl = ctx.enter_context(tc.tile_pool(name="g", bufs=8))
            vpp = ctx.enter_context(tc.tile_pool(name="vp", bufs=3))
            chp = ctx.enter_context(tc.tile_pool(name="ch", bufs=3))
            psp = ctx.enter_context(tc.tile_pool(name="ps", bufs=1, space="PSUM"))
            pop = ctx.enter_context(tc.tile_pool(name="po", bufs=2, space="PSUM"))
            otp = ctx.enter_context(tc.tile_pool(name="ot", bufs=3))
            gp2 = ctx.enter_context(tc.tile_pool(name="g2", bufs=6))
            ncalls = (NST + 1) // 2
            dpl = ctx.enter_context(
                tc.tile_pool(name="stg", bufs=ncalls, space="DRAM")
            )

            idxA_sb = cst.tile([128, VPC * 2 // 16], mybir.dt.int16)
            nc.sync.dma_start(out=idxA_sb[:], in_=idxA[:])
            idxB_sb = cst.tile([128, VPC * 2 // 16], mybir.dt.int16)
            nc.sync.dma_start(out=idxB_sb[:], in_=idxB[:])
            w_sb = []
            for j in range(4):
                wt = cst.tile([128, 128], F16, tag=f"w{j}", name=f"w{j}")
                w_sb.append(wt)
            for j in range(4):
                nc.sync.dma_start(out=w_sb[j][:], in_=wch[j])
            bias_sb = cst.tile([128, 1], F32)
            nc.sync.dma_start(out=bias_sb[:], in_=bias[:])
            ident = cst.tile([128, 128], F16)
            make_identity(nc, ident[:])

            def gather_call(c):
                nv = min(CALLV, VPC - c * CALLV)
                ni = nv * 2
                gA = gpl.tile([128, 16 * C], F32, tag="gA", name=f"gA{c}")
                gB = gpl.tile([128, 16 * C], F32, tag="gB", name=f"gB{c}")
                for g, tb, ix in ((gA, tblA, idxA_sb), (gB, tblB, idxB_sb)):
                    nc.gpsimd.dma_gather(
                        out_ap=g[:, : (ni // 128) * C].rearrange(
                            "p (k e) -> p k e", e=C
                        ),
                        in_ap=tb[:],
                        idxs_ap=ix[:, c * (CALLV * 2 // 16) : c * (CALLV * 2 // 16) + ni // 16],
                        num_idxs=ni,
                        num_idxs_reg=ni,
                        elem_size=C,
                        single_packet=False,
                    )
                return gA, gB

            def supertile(st, gA, gB):
                h4 = (st % 2) * 4
                # fp16 view: [128, kk(8), j(2), 128] with [0:64]=x, [64:128]=x^2
                grA = gA[:].bitcast(F16).rearrange("p (k j e) -> p k j e", k=8, j=2)
                grB = gB[:].bitcast(F16).rearrange("p (k j e) -> p k j e", k=8, j=2)
                f = [
                    grA[:, h4 : h4 + 4, 0, 0:64],
                    grA[:, h4 : h4 + 4, 1, 0:64],
                    grB[:, h4 : h4 + 4, 0, 0:64],
                    grB[:, h4 : h4 + 4, 1, 0:64],
                ]
                s = [
                    grA[:, h4 : h4 + 4, 0, 64:128],
                    grA[:, h4 : h4 + 4, 1, 64:128],
                    grB[:, h4 : h4 + 4, 0, 64:128],
                    grB[:, h4 : h4 + 4, 1, 64:128],
                ]

                vpA = vpp.tile([128, SG], F16, tag="vpA")  # [p1v | p2v]
                vpB = vpp.tile([128, SG], F16, tag="vpB")  # [p3v | mxdv]
                vpC = vpp.tile([128, SG], F16, tag="vpC")  # [p1v^2 | p1v^3]
                vpD = vpp.tile([128, SG // 2], F16, tag="vpD")  # [p1v*p2v]
                ta = vpp.tile([128, SG // 2], F16, tag="ta")
                tb = vpp.tile([128, SG // 2], F16, tag="tb")
                Ar = vpA[:].rearrange("p (k w) -> p k w", w=128)
                Br = vpB[:].rearrange("p (k w) -> p k w", w=128)
                Cr = vpC[:].rearrange("p (k w) -> p k w", w=128)
                Dr = vpD[:].rearrange("p (k w) -> p k w", w=64)
                tar = ta[:].rearrange("p (k w) -> p k w", w=64)
                tbr = tb[:].rearrange("p (k w) -> p k w", w=64)
                p1v, p2v = Ar[:, :, 0:64], Ar[:, :, 64:128]
                p3v, mxdv = Br[:, :, 0:64], Br[:, :, 64:128]
                tt = nc.vector.tensor_tensor
                op = mybir.AluOpType
                tt(out=p1v, in0=f[1], in1=f[2], op=op.add)
                tt(out=p1v, in0=p1v, in1=f[3], op=op.add)
                tt(out=p2v, in0=s[1], in1=s[2], op=op.add)
                tt(out=p2v, in0=p2v, in1=s[3], op=op.add)
                tt(out=tar, in0=f[1], in1=s[1], op=op.mult)
                tt(out=tbr, in0=f[2], in1=s[2], op=op.mult)
                tt(out=p3v, in0=tar, in1=tbr, op=op.add)
                tt(out=tar, in0=f[3], in1=s[3], op=op.mult)
                tt(out=p3v, in0=p3v, in1=tar, op=op.add)
                tt(out=tar, in0=f[1], in1=f[2], op=op.max)
                tt(out=tar, in0=tar, in1=f[3], op=op.max)
                tt(out=tbr, in0=f[1], in1=f[2], op=op.min)
                tt(out=tbr, in0=tbr, in1=f[3], op=op.min)
                tt(out=mxdv, in0=tar, in1=tbr, op=op.subtract)
                tt(out=Cr[:, :, 0:64], in0=p1v, in1=p1v, op=op.mult)
                tt(out=Cr[:, :, 64:128], in0=Cr[:, :, 0:64], in1=p1v, op=op.mult)
                tt(out=Dr[:, :, :], in0=p1v, in1=p2v, op=op.mult)

                psA = psp.tile([128, SG], F16, tag="psA")
                psB = psp.tile([128, SG], F16, tag="psB")
                psC = psp.tile([128, SG], F16, tag="psC")
                psD = psp.tile([128, SG], F16, tag="psD")
                for k in range(4):
                    sl = slice(k * 128, (k + 1) * 128)
                    nc.tensor.transpose(out=psA[:, sl], in_=vpA[:, sl], identity=ident[:])
                    nc.tensor.transpose(out=psB[:, sl], in_=vpB[:, sl], identity=ident[:])
                    nc.tensor.transpose(out=psC[:, sl], in_=vpC[:, sl], identity=ident[:])
                    nc.tensor.transpose(out=psD[0:64, sl], in_=Dr[:, k, :], identity=ident[:])
                    nc.tensor.transpose(
                        out=psD[64:128, sl], in_=f[0][:, k, :], identity=ident[:]
                    )

                chA = chp.tile([128, SG], F16, tag="chA")
                chB = chp.tile([128, SG], F16, tag="chB")
                chC = chp.tile([128, SG], F16, tag="chC")
                chD = chp.tile([128, SG], F16, tag="chD")
                nc.vector.tensor_copy(out=chA[:], in_=psA[:])
                nc.scalar.activation(
                    out=chB[:], in_=psB[:], func=mybir.ActivationFunctionType.Copy
                )
                nc.vector.tensor_copy(out=chC[:], in_=psC[:])
                nc.scalar.activation(
                    out=chD[:], in_=psD[:], func=mybir.ActivationFunctionType.Copy
                )

                psO = pop.tile([128, SG], F32, tag="psO")
                nc.tensor.matmul(out=psO[:], lhsT=w_sb[0][:], rhs=chA[:], start=True, stop=False)
                nc.tensor.matmul(out=psO[:], lhsT=w_sb[1][:], rhs=chB[:], start=False, stop=False)
                nc.tensor.matmul(out=psO[:], lhsT=w_sb[2][:], rhs=chC[:], start=False, stop=False)
                nc.tensor.matmul(out=psO[:], lhsT=w_sb[3][:], rhs=chD[:], start=False, stop=True)

                ot = otp.tile([128, SG], F16, tag="ot")
                nc.scalar.activation(
                    out=ot[:],
                    in_=psO[:],
                    func=mybir.ActivationFunctionType.Identity,
                    bias=bias_sb[:],
                )
                nc.sync.dma_start(out=out[:, st * SG : (st + 1) * SG], in_=ot[:])

            def body():
                if not two_phase:
                    for c in range((nst + 1) // 2):
                        gA, gB = gather_call(c)
                        for h2 in range(2):
                            st = 2 * c + h2
                            if st < nst:
                                supertile(st, gA, gB)
                    return
                # phase 1: pure gather -> DRAM staging (no compute-engine work,
                # so SWDGE descriptor generation runs unimpeded)
                stg = []
                for c in range((nst + 1) // 2):
                    gA, gB = gather_call(c)
                    sA = dpl.tile([128, 16 * C], F32, tag="sA", name=f"sA{c}")
                    sB = dpl.tile([128, 16 * C], F32, tag="sB", name=f"sB{c}")
                    nc.sync.dma_start(out=sA[:], in_=gA[:])
                    nc.sync.dma_start(out=sB[:], in_=gB[:])
                    stg.append((sA, sB))
                # phase 2: stream staged rows back contiguously + compute
                for c in range((nst + 1) // 2):
                    sA, sB = stg[c]
                    gA2 = gp2.tile([128, 16 * C], F32, tag="gA2", name=f"gA2_{c}")
                    gB2 = gp2.tile([128, 16 * C], F32, tag="gB2", name=f"gB2_{c}")
                    nc.sync.dma_start(out=gA2[:], in_=sA[:])
                    nc.sync.dma_start(out=gB2[:], in_=sB[:])
                    for h2 in range(2):
                        st = 2 * c + h2
                        if st < nst:
                            supertile(st, gA2, gB2)

            if loop_iters == 1:
                body()
            else:
                with tc.For_i(0, loop_iters, 1) as _:
                    body()

    nc.compile()
    _cache[key] = nc
    return nc


def prep_inputs(x, Gi, W, b):
    """Host-side sharding/packing. Returns list of 8 per-core input maps."""
    x = np.asarray(x)
    Gi = np.asarray(Gi)
    W = np.asarray(W, dtype=np.float32)
    b = np.asarray(b, dtype=np.float32)
    xs = x[..., 0].astype(np.float32)  # [B, C, V]

    # weight recombination (scale factors folded in)
    W0, W1, W2, W3, W4, W5, W6 = [W[:, :, k] for k in range(7)]  # each [CO, C]
    feats = {
        "p1": W1, "p2": W4 - W3 / 2, "p3": W6 + W2 / 3, "mxd": 2 * W5,
        "sq": W3 / 2, "cu": W2 / 6, "pp": -W2 / 2, "f0": W0,
    }
    pairs = [("p1", "p2"), ("p3", "mxd"), ("sq", "cu"), ("pp", "f0")]
    wch = np.zeros((4, 128, 128), dtype=np.float16)
    for j, (lo, hi) in enumerate(pairs):
        wch[j, 0:64, :] = feats[lo].T.astype(np.float16)
        wch[j, 64:128, :] = feats[hi].T.astype(np.float16)
    bias = b.reshape(128, 1).astype(np.float32)

    tbls = []
    for bb in range(B):
        x16 = np.ascontiguousarray(xs[bb].T).astype(np.float16)     # [V, C]
        sq16 = (x16.astype(np.float32) ** 2).astype(np.float16)     # [V, C]
        tbls.append(
            np.ascontiguousarray(np.concatenate([x16, sq16], axis=1)).view(np.float32)
        )

    def wrap16(inv2):
        # inv2: [VPC, 2] int ranks -> int16 SBUF layout [128, VPC*2//16]
        cols_out = []
        nc_ = (VPC + CALLV - 1) // CALLV
        for c in range(nc_):
            nv = min(CALLV, VPC - c * CALLV)
            blk = inv2[c * CALLV : c * CALLV + nv]             # [nv, 2]
            flat = (
                blk.reshape(nv // 128, 128, 2).transpose(0, 2, 1).reshape(-1)
            )                                                   # [ni]
            cols = flat.reshape(-1, 16).T                       # [16, ni//16]
            cols_out.append(np.tile(cols, (8, 1)))              # [128, ni//16]
        return np.ascontiguousarray(np.concatenate(cols_out, axis=1).astype(np.int16))

    maps = []
    for core in range(8):
        bb, h = divmod(core, 2)
        v0 = h * VPC
        nreal = min(VPC, V - v0)
        gi = np.zeros((VPC, 4), dtype=np.int64)
        gi[:nreal] = Gi[bb, v0 : v0 + nreal, :]
        m = {"wch": wch, "bias": bias}
        for nm, sl in (("A", slice(0, 2)), ("B", slice(2, 4))):
            u, inv = np.unique(gi[:, sl], return_inverse=True)
            assert len(u) < TBLR, len(u)
            tb = np.zeros((TBLR, C), dtype=np.float32)
            tb[: len(u)] = tbls[bb][u]
            m["tbl" + nm] = tb
            m["idx" + nm] = wrap16(inv.reshape(VPC, 2))
        maps.append(m)
    return maps


def assemble(results):
    out = np.zeros((B, CO, V, 1), dtype=np.float32)
    for core in range(8):
        bb, h = divmod(core, 2)
        v0 = h * VPC
        nreal = min(VPC, V - v0)
        o = results[core]["out"].astype(np.float32)  # [128, VPC]
        out[bb, :, v0 : v0 + nreal, 0] = o[:, :nreal]
    return out


def kernel(**inputs):
    nc = build_program(1)
    maps = prep_inputs(inputs["x"], inputs["Gi"], inputs["W"], inputs["b"])
    res = run_bass_kernel_spmd(nc, maps, list(range(8)))
    return assemble(res.results)



# revision 2
# speedup vs baseline: 1.5859x; 1.5859x over previous
"""MeshConvPoint Trainium2 kernel (8-core SPMD).

Math: per vertex v with gathered features f0..f3 (4 random indices/vertex):
  G = [f0, p1, e3, e2, p2, 2*(mx-mn), p3] channelwise over f1..f3,
  out = einsum(G, W) + b
where p_k = sum_j f_j^k, e2=(p1^2-p2)/2, e3=(p1^3-3 p1 p2 + 2 p3)/6,
mx/mn = max/min over f1..f3.  All symmetric functions reduce to the
8 features [f0, p1, p2, p3, p1^2, p1^3, p1*p2, mx-mn]; the linear
recombination + scale factors are folded into the weights host-side, so the
device does: gather -> power sums -> PE transposes -> one 512-contract
matmul per 512-vertex supertile.

Sharding: 8 cores = 4 batches x 2 vertex halves (data parallel; each core
holds the full per-batch gather table, so no collectives).

Gather: rows fetched with indirect DMA from a [TBLR, 64] f32-typed table
whose payload is bit-packed fp16 [x | x^2] (squares ride along for free since
the gather cost is per-descriptor, not per-byte).  Compute reads the gather
tiles directly (no DRAM staging round-trip); descriptor generation on the
Pool engine free-runs ahead of compute, which hides the entire elementwise/
transpose/matmul phase under the gather.
"""

import sys

sys.path.insert(0, "/opt/trn_rl_repo")

import numpy as np

import concourse.bass as bass
import concourse.tile as tile
from concourse import bacc, mybir
from concourse.bass_utils import run_bass_kernel_spmd
from concourse.masks import make_identity

B, C, V, CO, K = 4, 64, 50000, 128, 7
VPC = 25088          # padded vertices per core (2 halves of 50000 -> 196*128)
SG = 512             # supertile = 4 vtiles of 128 vertices
NST = VPC // SG      # 49 supertiles
TBLR = 32768         # fixed compacted table rows (unique refs per slot-pair < 32768)
CALLS = (2048,) * 12 + (512,)   # vertices per dma_gather call
F16 = mybir.dt.float16
F32 = mybir.dt.float32

_cache = {}


def build_program(loop_iters=1, calls=CALLS, bufs=4):
    key = (loop_iters, tuple(calls), bufs)
    if key in _cache:
        return _cache[key]
    assert sum(calls) == VPC and all(c % SG == 0 for c in calls)
    nc = bacc.Bacc("TRN2", target_bir_lowering=False, debug=False, num_devices=8)
    tblA = nc.dram_tensor("tblA", [TBLR, C], F32, kind="ExternalInput").ap()
    tblB = nc.dram_tensor("tblB", [TBLR, C], F32, kind="ExternalInput").ap()
    idxA = nc.dram_tensor("idxA", [128, VPC * 2 // 16], mybir.dt.int16, kind="ExternalInput").ap()
    idxB = nc.dram_tensor("idxB", [128, VPC * 2 // 16], mybir.dt.int16, kind="ExternalInput").ap()
    wch = nc.dram_tensor("wch", [4, 128, 128], F16, kind="ExternalInput").ap()
    bias = nc.dram_tensor("bias", [128, 1], F32, kind="ExternalInput").ap()
    out = nc.dram_tensor("out", [128, NST * SG], F16, kind="ExternalOutput").ap()

    with tile.TileContext(nc) as tc:
        import contextlib

        with contextlib.ExitStack() as ctx:
            cst = ctx.enter_context(tc.tile_pool(name="cst", bufs=1))
            gpl = ctx.enter_context(tc.tile_pool(name="g", bufs=bufs))
            vpp = ctx.enter_context(tc.tile_pool(name="vp", bufs=3))
            chp = ctx.enter_context(tc.tile_pool(name="ch", bufs=3))
            psp = ctx.enter_context(tc.tile_pool(name="ps", bufs=1, space="PSUM"))
            pop = ctx.enter_context(tc.tile_pool(name="po", bufs=2, space="PSUM"))
            otp = ctx.enter_context(tc.tile_pool(name="ot", bufs=3))

            idxA_sb = cst.tile([128, VPC * 2 // 16], mybir.dt.int16)
            nc.sync.dma_start(out=idxA_sb[:], in_=idxA[:])
            idxB_sb = cst.tile([128, VPC * 2 // 16], mybir.dt.int16)
            nc.sync.dma_start(out=idxB_sb[:], in_=idxB[:])
            w_sb = []
            for j in range(4):
                wt = cst.tile([128, 128], F16, tag=f"w{j}", name=f"w{j}")
                w_sb.append(wt)
            for j in range(4):
                nc.sync.dma_start(out=w_sb[j][:], in_=wch[j])
            bias_sb = cst.tile([128, 1], F32)
            nc.sync.dma_start(out=bias_sb[:], in_=bias[:])
            ident = cst.tile([128, 128], F16)
            make_identity(nc, ident[:])

            def gather_call(c, v0, nv):
                ni = nv * 2
                gA = gpl.tile([128, (ni // 128) * C], F32, tag="gA", name=f"gA{c}")
                gB = gpl.tile([128, (ni // 128) * C], F32, tag="gB", name=f"gB{c}")
                for g, tb, ix in ((gA, tblA, idxA_sb), (gB, tblB, idxB_sb)):
                    nc.gpsimd.dma_gather(
                        out_ap=g[:].rearrange("p (k e) -> p k e", e=C),
                        in_ap=tb[:],
                        idxs_ap=ix[:, v0 * 2 // 16 : v0 * 2 // 16 + ni // 16],
                        num_idxs=ni,
                        num_idxs_reg=ni,
                        elem_size=C,
                        single_packet=False,
                    )
                return gA, gB

            def supertile(st, gA, gB, kgroups, stloc):
                h4 = stloc * 4
                grA = gA[:].bitcast(F16).rearrange("p (k j e) -> p k j e", k=kgroups, j=2)
                grB = gB[:].bitcast(F16).rearrange("p (k j e) -> p k j e", k=kgroups, j=2)
                f = [
                    grA[:, h4 : h4 + 4, 0, 0:64],
                    grA[:, h4 : h4 + 4, 1, 0:64],
                    grB[:, h4 : h4 + 4, 0, 0:64],
                    grB[:, h4 : h4 + 4, 1, 0:64],
                ]
                s = [
                    grA[:, h4 : h4 + 4, 0, 64:128],
                    grA[:, h4 : h4 + 4, 1, 64:128],
                    grB[:, h4 : h4 + 4, 0, 64:128],
                    grB[:, h4 : h4 + 4, 1, 64:128],
                ]

                vpA = vpp.tile([128, SG], F16, tag="vpA")  # [p1v | p2v]
                vpB = vpp.tile([128, SG], F16, tag="vpB")  # [p3v | mxdv]
                vpC = vpp.tile([128, SG], F16, tag="vpC")  # [p1v^2 | p1v^3]
                vpD = vpp.tile([128, SG // 2], F16, tag="vpD")  # [p1v*p2v]
                ta = vpp.tile([128, SG // 2], F16, tag="ta")
                tb = vpp.tile([128, SG // 2], F16, tag="tb")
                Ar = vpA[:].rearrange("p (k w) -> p k w", w=128)
                Br = vpB[:].rearrange("p (k w) -> p k w", w=128)
                Cr = vpC[:].rearrange("p (k w) -> p k w", w=128)
                Dr = vpD[:].rearrange("p (k w) -> p k w", w=64)
                tar = ta[:].rearrange("p (k w) -> p k w", w=64)
                tbr = tb[:].rearrange("p (k w) -> p k w", w=64)
                p1v, p2v = Ar[:, :, 0:64], Ar[:, :, 64:128]
                p3v, mxdv = Br[:, :, 0:64], Br[:, :, 64:128]
                tt = nc.vector.tensor_tensor
                op = mybir.AluOpType
                tt(out=p1v, in0=f[1], in1=f[2], op=op.add)
                tt(out=p1v, in0=p1v, in1=f[3], op=op.add)
                tt(out=p2v, in0=s[1], in1=s[2], op=op.add)
                tt(out=p2v, in0=p2v, in1=s[3], op=op.add)
                tt(out=tar, in0=f[1], in1=s[1], op=op.mult)
                tt(out=tbr, in0=f[2], in1=s[2], op=op.mult)
                tt(out=p3v, in0=tar, in1=tbr, op=op.add)
                tt(out=tar, in0=f[3], in1=s[3], op=op.mult)
                tt(out=p3v, in0=p3v, in1=tar, op=op.add)
                tt(out=tar, in0=f[1], in1=f[2], op=op.max)
                tt(out=tar, in0=tar, in1=f[3], op=op.max)
                tt(out=tbr, in0=f[1], in1=f[2], op=op.min)
                tt(out=tbr, in0=tbr, in1=f[3], op=op.min)
                tt(out=mxdv, in0=tar, in1=tbr, op=op.subtract)
                tt(out=Cr[:, :, 0:64], in0=p1v, in1=p1v, op=op.mult)
                tt(out=Cr[:, :, 64:128], in0=Cr[:, :, 0:64], in1=p1v, op=op.mult)
                tt(out=Dr[:, :, :], in0=p1v, in1=p2v, op=op.mult)

                psA = psp.tile([128, SG], F16, tag="psA")
                psB = psp.tile([128, SG], F16, tag="psB")
                psC = psp.tile([128, SG], F16, tag="psC")
                psD = psp.tile([128, SG], F16, tag="psD")
                for k in range(4):
                    sl = slice(k * 128, (k + 1) * 128)
                    nc.tensor.transpose(out=psA[:, sl], in_=vpA[:, sl], identity=ident[:])
                    nc.tensor.transpose(out=psB[:, sl], in_=vpB[:, sl], identity=ident[:])
                    nc.tensor.transpose(out=psC[:, sl], in_=vpC[:, sl], identity=ident[:])
                    nc.tensor.transpose(out=psD[0:64, sl], in_=Dr[:, k, :], identity=ident[:])
                    nc.tensor.transpose(
                        out=psD[64:128, sl], in_=f[0][:, k, :], identity=ident[:]
                    )

                chA = chp.tile([128, SG], F16, tag="chA")
                chB = chp.tile([128, SG], F16, tag="chB")
                chC = chp.tile([128, SG], F16, tag="chC")
                chD = chp.tile([128, SG], F16, tag="chD")
                nc.vector.tensor_copy(out=chA[:], in_=psA[:])
                nc.scalar.activation(
                    out=chB[:], in_=psB[:], func=mybir.ActivationFunctionType.Copy
                )
                nc.vector.tensor_copy(out=chC[:], in_=psC[:])
                nc.scalar.activation(
                    out=chD[:], in_=psD[:], func=mybir.ActivationFunctionType.Copy
                )

                psO = pop.tile([128, SG], F32, tag="psO")
                nc.tensor.matmul(out=psO[:], lhsT=w_sb[0][:], rhs=chA[:], start=True, stop=False)
                nc.tensor.matmul(out=psO[:], lhsT=w_sb[1][:], rhs=chB[:], start=False, stop=False)
                nc.tensor.matmul(out=psO[:], lhsT=w_sb[2][:], rhs=chC[:], start=False, stop=False)
                nc.tensor.matmul(out=psO[:], lhsT=w_sb[3][:], rhs=chD[:], start=False, stop=True)

                ot = otp.tile([128, SG], F16, tag="ot")
                nc.scalar.activation(
                    out=ot[:],
                    in_=psO[:],
                    func=mybir.ActivationFunctionType.Identity,
                    bias=bias_sb[:],
                )
                nc.sync.dma_start(out=out[:, st * SG : (st + 1) * SG], in_=ot[:])

            def body():
                offs, v0 = [], 0
                for nv in calls:
                    offs.append(v0)
                    v0 += nv
                tiles = []
                for c in range(len(calls)):
                    tiles.append(gather_call(c, offs[c], calls[c]))
                st = 0
                for c in range(len(calls)):
                    gA, gB = tiles[c]
                    for h in range(calls[c] // SG):
                        supertile(st, gA, gB, calls[c] // 128, h)
                        st += 1

            if loop_iters == 1:
                body()
            else:
                with tc.For_i(0, loop_iters, 1) as _:
                    body()

    nc.compile()
    _cache[key] = nc
    return nc


def prep_inputs(x, Gi, W, b):
    """Host-side sharding/packing. Returns list of 8 per-core input maps."""
    x = np.asarray(x)
    Gi = np.asarray(Gi)
    W = np.asarray(W, dtype=np.float32)
    b = np.asarray(b, dtype=np.float32)
    xs = x[..., 0].astype(np.float32)  # [B, C, V]

    # weight recombination (scale factors folded in)
    W0, W1, W2, W3, W4, W5, W6 = [W[:, :, k] for k in range(7)]  # each [CO, C]
    feats = {
        "p1": W1, "p2": W4 - W3 / 2, "p3": W6 + W2 / 3, "mxd": 2 * W5,
        "sq": W3 / 2, "cu": W2 / 6, "pp": -W2 / 2, "f0": W0,
    }
    pairs = [("p1", "p2"), ("p3", "mxd"), ("sq", "cu"), ("pp", "f0")]
    wch = np.zeros((4, 128, 128), dtype=np.float16)
    for j, (lo, hi) in enumerate(pairs):
        wch[j, 0:64, :] = feats[lo].T.astype(np.float16)
        wch[j, 64:128, :] = feats[hi].T.astype(np.float16)
    bias = b.reshape(128, 1).astype(np.float32)

    tbls = []
    for bb in range(B):
        x16 = np.ascontiguousarray(xs[bb].T).astype(np.float16)     # [V, C]
        sq16 = (x16.astype(np.float32) ** 2).astype(np.float16)     # [V, C]
        tbls.append(
            np.ascontiguousarray(np.concatenate([x16, sq16], axis=1)).view(np.float32)
        )

    def wrap16(inv2):
        # inv2: [VPC, 2] int ranks -> int16 SBUF layout [128, VPC*2//16];
        # per 128-vertex group: [slot0 x 128, slot1 x 128] (call-size agnostic)
        flat = inv2.reshape(VPC // 128, 128, 2).transpose(0, 2, 1).reshape(-1)
        cols = flat.reshape(-1, 16).T                               # [16, VPC*2//16]
        return np.ascontiguousarray(np.tile(cols, (8, 1)).astype(np.int16))

    maps = []
    for core in range(8):
        bb, h = divmod(core, 2)
        v0 = h * VPC
        nreal = min(VPC, V - v0)
        gi = np.zeros((VPC, 4), dtype=np.int64)
        gi[:nreal] = Gi[bb, v0 : v0 + nreal, :]
        m = {"wch": wch, "bias": bias}
        for nm, sl in (("A", slice(0, 2)), ("B", slice(2, 4))):
            u, inv = np.unique(gi[:, sl], return_inverse=True)
            assert len(u) < TBLR, len(u)
            tb = np.zeros((TBLR, C), dtype=np.float32)
            tb[: len(u)] = tbls[bb][u]
            m["tbl" + nm] = tb
            m["idx" + nm] = wrap16(inv.reshape(VPC, 2))
        maps.append(m)
    return maps


def assemble(results):
    out = np.zeros((B, CO, V, 1), dtype=np.float32)
    for core in range(8):
        bb, h = divmod(core, 2)
        v0 = h * VPC
        nreal = min(VPC, V - v0)
        o = results[core]["out"].astype(np.float32)  # [128, VPC]
        out[bb, :, v0 : v0 + nreal, 0] = o[:, :nreal]
    return out


def kernel(**inputs):
    nc = build_program(1)
    maps = prep_inputs(inputs["x"], inputs["Gi"], inputs["W"], inputs["b"])
    res = run_bass_kernel_spmd(nc, maps, list(range(8)))
    return assemble(res.results)
